# revision 61
# baseline (speedup 1.0000x reference)
"""Multi-head causal attention (B=8, S=1024, D=768, H=12) on 8 trn2 NeuronCores.

Strategy: data-parallel over batch (one batch element per core, no collectives).

Per-core dataflow (all matmuls bf16 into fp32 PSUM):
  - host passes x^T and all weights pre-cast to bf16; Q^T/K^T via transposed
    projection (W stationary, x^T moving), V via natural projection (x^T
    stationary, W_v moving) -> no on-device transposes.
  - attention as S^T[k,q] = K @ Q^T per head; the two heads of a 128-row
    group go to the two 512-column halves of one PSUM tile (tile_position
    row packing).
  - causal handling: for diagonal-crossing key blocks the fully-masked low
    query columns are skipped in BOTH the QK^T and A@V matmuls (N-width
    trim); exp covers only the valid span of BOTH heads in ONE strided
    activation, and just the [128,2,128] diagonal triangle pair gets a
    single bf16 mask multiply (in place).
  - kc-granular software pipeline: QK^T(kc) ... A@V(kc-2) keeps the PE fed
    while ScalarE exponentiates; Q/K-proj, V-proj and output-proj work units
    are woven into the remaining gaps (matmuls first, PSUM->SBUF finish
    copies deferred to the next slot) so the PE never idles and its p-state
    stays at max clock.
  - startup: DMA order interleaves xT/wv chunk pairs with the nt=0 blocks
    of wk/wq so the stream can start ~10us in; the warm-up V/Q/K units are
    split into two 3-chunk contraction passes (partial to SBUF bf16, then
    in-place add) so the in-order PE is never serialized behind the last
    xT chunk.
  - softmax: exp straight out of PSUM (1/8 scale folded into W_q host-side;
    scores are small, no max-subtraction); denominator free via a ones
    column appended to V (row 64 of the A@V PSUM); fast reciprocal from
    PSUM on DVE; partition broadcast via a K=1 matmul; division on DVE
    into out^T (rows 64-127 via a small tile + partition-shifting DMA).
  - tail: SE(2,3) woven into the qc=1 stream; the last group's denominator
    chain is hidden behind SE st4/st5 partial accumulations; final copies
    and y DMAs split per 512/256 half.
"""
import sys

if "/opt/trn_rl_repo" not in sys.path:
    sys.path.insert(0, "/opt/trn_rl_repo")

import numpy as np

B, S, D, H = 8, 1024, 768, 12
DH = 64
NC_ = 8
NT = D // 128    # 6
ST = S // 128    # 8
QC = S // 512    # 2
# per-head V slot padded to 128 cols (col 64 = softmax-denominator ones,
# 65-127 zero) so the A@V stationary is a full 128-col FWL-eligible load
VPW = H * 128  # 1536

_compiled = None


def _build_masks():
    # [128, 2, 128] lower-tri mask for the diagonal 128x128 block (the
    # triangle is t-independent), duplicated across the two packed heads
    import ml_dtypes

    i = np.arange(128)[:, None]
    j = np.arange(128)[None, :]
    m = (i <= j).astype(np.float32)
    m = np.broadcast_to(m[:, None, :], (128, 2, 128))
    return np.ascontiguousarray(m).astype(ml_dtypes.bfloat16)


def _build_nc():
    import concourse.bass as bass
    import concourse.mybir as mybir
    import concourse.tile as tile
    from concourse import bacc

    F32 = mybir.dt.float32
    F32R = mybir.dt.float32r
    BF16 = mybir.dt.bfloat16
    AF = mybir.ActivationFunctionType
    MULT = mybir.AluOpType.mult
    ADD = mybir.AluOpType.add

    nc = bacc.Bacc("TRN2", target_bir_lowering=False, debug=False)

    # inputs arrive host-reordered to the exact SBUF layout
    xT_d = nc.dram_tensor("xT", [128, NT * S], BF16, kind="ExternalInput")
    wq_d = nc.dram_tensor("wq", [128, NT * D], BF16, kind="ExternalInput")
    wk_d = nc.dram_tensor("wk", [128, NT * D], BF16, kind="ExternalInput")
    wv_d = nc.dram_tensor("wv", [128, NT * D], BF16, kind="ExternalInput")
    wp_d = nc.dram_tensor("wp", [128, NT * D], BF16, kind="ExternalInput")
    mask_d = nc.dram_tensor("masks", [128, 2, 128], BF16,
                            kind="ExternalInput")
    # bf16 output: halves the tail output-DMA drain; host upcasts
    y_d = nc.dram_tensor("y", [S, D], BF16, kind="ExternalOutput")

    with tile.TileContext(nc) as tc:
        with (
            tc.tile_pool(name="static", bufs=1) as static,
            tc.tile_pool(name="pt", bufs=8) as ptp,
            tc.tile_pool(name="pu", bufs=4) as pup,
            tc.tile_pool(name="rr", bufs=4) as rrp,
            tc.tile_pool(name="dv", bufs=4) as dvp,
            tc.tile_pool(name="ysb", bufs=2) as ysbp,
            tc.tile_pool(name="psb", bufs=2, space="PSUM") as psb,
            tc.tile_pool(name="po", bufs=2, space="PSUM") as pop,
            tc.tile_pool(name="psh", bufs=2, space="PSUM") as psh,
        ):
            # ---- persistent SBUF ----
            xT = static.tile([128, NT, S], BF16)
            qT = static.tile([128, NT, S], BF16)
            kT = static.tile([128, NT, S], BF16)
            vp = static.tile([128, ST, VPW], BF16)
            # out^T split per qc half so tail SE units don't pick up false
            # whole-tile dependencies on the last group's division writes
            outT0 = static.tile([128, NT, 512], BF16)
            outT1 = static.tile([128, NT, 512], BF16)
            outTq = (outT0, outT1)
            msk = static.tile([128, 2, 128], BF16)
            wv_sb = static.tile([128, NT, D], BF16)
            # wk/wq nt-blocked: [p, nt, dc, 128] so one DMA lands a full
            # PJ stationary column block
            wk_sb = static.tile([128, NT, NT, 128], BF16)
            wq_sb = static.tile([128, NT, NT, 128], BF16)
            wp_sb = static.tile([128, NT, D], BF16)

            # ones columns of vp ride along A@V as the softmax denominator;
            # the FWL pad (cols 65-127) is zeroed so PSUM partitions 65-127
            # stay clean
            vp_h = vp[:].rearrange("p s (h e) -> p s h e", e=128)
            nc.vector.memset(vp_h[:, :, :, DH + 1:], 0.0)
            nc.gpsimd.memset(vp_h[:, :, :, DH:DH + 1], 1.0)
            # 1x64 ones row at partition 64: K=1 matmul broadcasts the den
            # row of pu to PSUM partitions 0-63 (no DMA round-trip)
            onesb = static.tile([128, 64], F32)
            nc.vector.memset(onesb[:], 1.0)

            # ---- input DMA order (one HWDGE queue saturates HBM; order =
            # consumption order): xT/wv chunk pairs, nt=0 weight blocks
            # early so the stream can start, masks, rest, wp last ----
            def dma_xv(dc):
                nc.sync.dma_start(xT[:, dc, :], xT_d[:, S * dc:S * (dc + 1)])
                nc.sync.dma_start(wv_sb[:, dc, :], wv_d[:, D * dc:D * (dc + 1)])

            dma_xv(0)
            dma_xv(1)
            dma_xv(2)
            nc.sync.dma_start(wk_sb[:, 0, :, :], wk_d[:, 0:D])
            nc.sync.dma_start(wq_sb[:, 0, :, :], wq_d[:, 0:D])
            dma_xv(3)
            dma_xv(4)
            dma_xv(5)
            nc.sync.dma_start(msk[:], mask_d[:])
            for nt in range(1, NT):
                nc.sync.dma_start(wk_sb[:, nt, :, :],
                                  wk_d[:, D * nt:D * (nt + 1)])
                nc.sync.dma_start(wq_sb[:, nt, :, :],
                                  wq_d[:, D * nt:D * (nt + 1)])
            nc.sync.dma_start(
                wp_sb[:].rearrange("p a b -> p (a b)"), wp_d[:, :])

            # ---- work units: emit matmuls now, return a finish closure ----
            def VP(st, half, fin_eng):
                ps = psh.tile([128, 512], F32, tag="psh", name=f"vps{st}h{half}")
                c0 = 512 * half
                w = 512 if half == 0 else 256
                for dc in range(NT):
                    nc.tensor.matmul(
                        ps[:, 0:w], xT[:, dc, 128 * st:128 * (st + 1)],
                        wv_sb[:, dc, c0:c0 + w],
                        start=(dc == 0), stop=(dc == NT - 1))

                dst = vp[:, st, :].rearrange("p (h e) -> p h e", e=128)

                def fin():
                    src = ps[:, 0:w].rearrange("p (h d) -> p h d", d=DH)
                    if half == 0:
                        o = dst[:, 0:8, 0:DH]
                    else:
                        o = dst[:, 8:12, 0:DH]
                    if fin_eng == "s":
                        nc.scalar.activation(o, src, AF.Copy)
                    else:
                        nc.vector.tensor_copy(out=o, in_=src)
                return fin

            def PJ(w_sb, dstT, nt, sc, fin_eng):
                ps = psh.tile([128, 512], F32, tag="psh", name=f"pj{nt}_{sc}")
                for dc in range(NT):
                    nc.tensor.matmul(
                        ps[:], w_sb[:, nt, dc, :],
                        xT[:, dc, 512 * sc:512 * (sc + 1)],
                        start=(dc == 0), stop=(dc == NT - 1))

                def fin():
                    o = dstT[:, nt, 512 * sc:512 * (sc + 1)]
                    if fin_eng == "s":
                        nc.scalar.activation(o, ps[:], AF.Copy)
                    else:
                        nc.vector.tensor_copy(out=o, in_=ps[:])
                return fin

            # ---- split warm-up units: two 3-chunk contraction passes so
            # the PE isn't serialized behind the last xT chunk during the
            # input DMA window. Partial lands in the bf16 destination, the
            # second pass adds the PSUM half in place on DVE. ----
            def VPw_pass(st, half, p):
                ps = psh.tile([128, 512], F32, tag="psh",
                              name=f"vw{st}h{half}p{p}")
                c0 = 512 * half
                w = 512 if half == 0 else 256
                dcs = (0, 1, 2) if p == 0 else (3, 4, 5)
                for i, dc in enumerate(dcs):
                    nc.tensor.matmul(
                        ps[:, 0:w], xT[:, dc, 128 * st:128 * (st + 1)],
                        wv_sb[:, dc, c0:c0 + w],
                        start=(i == 0), stop=(i == 2))
                dst = vp[:, st, :].rearrange("p (h e) -> p h e", e=128)
                src = ps[:, 0:w].rearrange("p (h d) -> p h d", d=DH)
                if half == 0:
                    o = dst[:, 0:8, 0:DH]
                else:
                    o = dst[:, 8:12, 0:DH]
                if p == 0:
                    nc.vector.tensor_copy(out=o, in_=src)
                else:
                    nc.vector.tensor_tensor(o, src, o, ADD)

            def PJw_pass(w_sb, dstT, nt, sc, p):
                ps = psh.tile([128, 512], F32, tag="psh",
                              name=f"pw{nt}_{sc}p{p}")
                dcs = (0, 1, 2) if p == 0 else (3, 4, 5)
                for i, dc in enumerate(dcs):
                    nc.tensor.matmul(
                        ps[:], w_sb[:, nt, dc, :],
                        xT[:, dc, 512 * sc:512 * (sc + 1)],
                        start=(i == 0), stop=(i == 2))
                o = dstT[:, nt, 512 * sc:512 * (sc + 1)]
                if p == 0:
                    nc.vector.tensor_copy(out=o, in_=ps[:])
                else:
                    nc.vector.tensor_tensor(o, ps[:], o, ADD)

            ysb_tiles = {}

            def SE(st, half, fin_eng):
                if half == 0:
                    ysb_tiles[st] = ysbp.tile([128, D], BF16, tag="ysb",
                                              name=f"ysb{st}")
                ysb = ysb_tiles[st]
                ps = psh.tile([128, 512], F32, tag="psh", name=f"se{st}h{half}")
                c0 = 512 * half
                w = 512 if half == 0 else 256
                oT = outTq[st // 4]
                oc = 128 * (st % 4)
                for dc in range(NT):
                    nc.tensor.matmul(
                        ps[:, 0:w], oT[:, dc, oc:oc + 128],
                        wp_sb[:, dc, c0:c0 + w],
                        start=(dc == 0), stop=(dc == NT - 1))

                def fin():
                    if fin_eng == "s":
                        nc.scalar.activation(ysb[:, c0:c0 + w], ps[:, 0:w],
                                             AF.Copy)
                    else:
                        nc.vector.tensor_copy(out=ysb[:, c0:c0 + w],
                                              in_=ps[:, 0:w])
                    if half == 1:
                        nc.sync.dma_start(y_d[128 * st:128 * (st + 1), :],
                                          ysb[:])
                return fin

            # ---- filler scheduler ----
            fillers = []
            pend_fin = []

            def flush_fins():
                while pend_fin:
                    pend_fin.pop(0)()

            def pop_fill(n=1):
                for _ in range(n):
                    flush_fins()
                    if fillers:
                        fin = fillers.pop(0)[1]()
                        pend_fin.append(fin)

            def force(*keys):
                # deadline path: unit's finish copy must land before the
                # consumer instructions that follow, so emit it immediately
                for key in keys:
                    for i, (k, fn) in enumerate(fillers):
                        if k == key:
                            flush_fins()
                            fillers.pop(i)
                            fn()()
                            break

            def mk_vp(st, half, e):
                return (f"vp{st}h{half}", lambda: VP(st, half, e))

            def mk_pj(kind, nt, sc, e):
                if kind == "q":
                    return (f"q{nt}s{sc}", lambda: PJ(wq_sb, qT, nt, sc, e))
                return (f"k{nt}s{sc}", lambda: PJ(wk_sb, kT, nt, sc, e))

            def mk_se(st, half, e):
                return (f"se{st}h{half}", lambda: SE(st, half, e))

            # ---- attention helpers ----
            pend_rec = []   # (hp, qc, pu): needs den-broadcast mm + recip
            pend_den = []   # (hp, qc, pu, rr0, rr1): needs division

            def flush_rec(pe=False):
                while pend_rec:
                    hp_, qc_, pu_ = pend_rec.pop(0)
                    rr = rrp.tile([64, 1024], F32, tag="rr", name="rr")
                    # K=1 matmul broadcasts the den row of pu to PSUM
                    # partitions 0-63 (no DRAM round-trip)
                    for hh in range(2):
                        denb = psh.tile([128, 512], F32, tag="psh",
                                        name=f"denb{hh}")
                        nc.tensor.matmul(
                            denb[0:64, :], onesb[64:65, :].bitcast(F32R),
                            pu_[64:65, 512 * hh:512 * (hh + 1)],
                            start=True, stop=True, tile_position=(64, 0))
                        nc.vector.reciprocal_approx_fast(
                            out=rr[:, 512 * hh:512 * (hh + 1)],
                            in_=denb[0:64, :])
                    pend_den.append((hp_, qc_, pu_,
                                     rr[:, 0:512], rr[:, 512:1024]))

            def flush_den():
                while pend_den:
                    hp_, qc_, pu_, rr0, rr1 = pend_den.pop(0)
                    oT = outTq[qc_]
                    nc.vector.tensor_tensor(
                        oT[0:64, hp_, :],
                        pu_[0:64, 0:512], rr0, MULT)
                    # DVE lanes are partition-locked: rows 64-127 go via a
                    # small tile + partition-shifting local DMA
                    dv = dvp.tile([64, 512], BF16, tag="dv", name="dv")
                    nc.vector.tensor_tensor(dv[:], pu_[0:64, 512:1024],
                                            rr1, MULT)
                    nc.sync.dma_start(oT[64:128, hp_, :], dv[:])

            # ---- the attention kc-stream ----
            gstate = {}

            def qkt_g(qc, hp, kc):
                g = gstate.setdefault((qc, hp), {"pts": {}, "offs": {},
                                                 "po": {}})
                t = kc - 4 * qc
                off = 128 * t if 0 <= t <= 3 else 0
                g["offs"][kc] = off
                ps = psb.tile([128, 1024], F32, tag="big",
                              name=f"s_{qc}_{hp}_{kc}")
                for hh in range(2):
                    rows = slice(64 * hh, 64 * (hh + 1))
                    nc.tensor.matmul(
                        ps[:, 512 * hh + off:512 * (hh + 1)],
                        kT[rows, hp, 128 * kc:128 * (kc + 1)],
                        qT[rows, hp, 512 * qc + off:512 * (qc + 1)],
                        start=True, stop=True,
                        tile_position=(64 * hh, 0))
                pt = ptp.tile([128, 1024], BF16, tag="pt")
                if 0 <= t <= 3:
                    # one strided activation covers both heads' valid span;
                    # one strided tensor_tensor masks both diagonal triangles
                    pv = pt[:].rearrange("p (h q) -> p h q", h=2)
                    sv = ps[:].rearrange("p (h q) -> p h q", h=2)
                    nc.scalar.activation(pv[:, :, off:512], sv[:, :, off:512],
                                         AF.Exp)
                    tri = slice(off, off + 128)
                    nc.vector.tensor_tensor(
                        pv[:, :, tri], pv[:, :, tri], msk[:], MULT)
                else:
                    nc.scalar.activation(pt[:], ps[:], AF.Exp)
                g["pts"][kc] = [(pt, 0), (pt, 512)]

            def av_g(qc, hp, kc):
                g = gstate[(qc, hp)]
                K = 4 * (qc + 1)
                off = g["offs"][kc]
                for hh in range(2):
                    if hh not in g["po"]:
                        g["po"][hh] = pop.tile([128, 512], F32, tag="po",
                                               name=f"po_{qc}_{hp}_{hh}")
                    h = 2 * hp + hh
                    src, c0 = g["pts"][kc][hh]
                    nc.tensor.matmul(
                        g["po"][hh][:, off:512],
                        vp[:, kc, 128 * h:128 * (h + 1)],
                        src[:, c0 + off:c0 + 512],
                        start=(kc == 0), stop=(kc == K - 1))
                if kc == K - 1:
                    pu = pup.tile([65, 1024], F32R, tag="pu")
                    for hh in range(2):
                        nc.vector.tensor_copy(
                            out=pu[:, 512 * hh:512 * (hh + 1)],
                            in_=g["po"][hh][0:65, :])
                    pend_rec.append((hp, qc, pu))
                    del gstate[(qc, hp)]

            # ---- phase A: split warm-up (VP st0-3 + k00/k01/q00) keeps the
            # PE as busy as the chunked input DMA allows ----
            warm = [("v", 0, 0), ("v", 0, 1), ("k", 0, 0), ("v", 1, 0),
                    ("v", 1, 1), ("q", 0, 0), ("v", 2, 0), ("v", 2, 1),
                    ("k", 0, 1), ("v", 3, 0), ("v", 3, 1)]
            for p in (0, 1):
                for kind, a, b in warm:
                    if kind == "v":
                        VPw_pass(a, b, p)
                    elif kind == "k":
                        PJw_pass(wk_sb, kT, a, b, p)
                    else:
                        PJw_pass(wq_sb, qT, a, b, p)

            # VP units first: their wv/xT inputs land earliest, and (1,0)
            # -- the third group -- already consumes vp st4-7
            fillers += [mk_vp(st, h, "s") for st in (4, 5, 6, 7)
                        for h in (0, 1)]
            for nt in range(1, NT):
                fillers += [mk_pj("q", nt, 0, "s"), mk_pj("k", nt, 0, "s"),
                            mk_pj("k", nt, 1, "s")]
            fillers += [mk_pj("q", nt, 1, "v") for nt in range(NT)]

            # ---- the stream: qc0/qc1 groups interleaved so the ScalarE
            # exp load (3.3us per qc0 group vs 7.9us per qc1 group) is
            # spread evenly -- otherwise the 2-buf score-PSUM pool couples
            # QK^T(s+2) to exp(s) and the PE stalls through the qc1 half ----
            LAG = 3
            slot0 = {0: 1, 1: 1, 2: 1, 3: 1}
            slot1 = {0: 1, 2: 1, 4: 1, 6: 1}
            groups = [(0, 0), (0, 1), (1, 0), (0, 2), (1, 1), (0, 3),
                      (1, 2), (0, 4), (1, 3), (0, 5), (1, 4), (1, 5)]
            steps = [(qc, hp, kc)
                     for qc, hp in groups
                     for kc in range(4 * (qc + 1))]
            rec_due = []   # stream indices: den-broadcast mm + reciprocal
            den_due = []   # stream indices: divisions (2 steps later)
            gi = -1
            for s, (qc, hp, kc) in enumerate(steps):
                if kc == 0:
                    gi += 1
                    if qc == 0 and hp > 0:
                        force(f"q{hp}s0", f"k{hp}s0")
                    elif qc == 1:
                        force(f"q{hp}s0", f"k{hp}s0", f"k{hp}s1",
                              f"q{hp}s1")
                        # A@V kc>=4 reads vp st4-7: make sure the VP
                        # fillers have run (no-op once popped)
                        force(*[f"vp{st}h{h}" for st in (4, 5, 6, 7)
                                for h in (0, 1)])
                    if gi == 10:
                        # all qc0 groups divided by now: SE st0-3 fill the
                        # last two (qc1) groups
                        fillers += [mk_se(st, h, "v") for st in (0, 1, 2, 3)
                                    for h in (0, 1)]
                qkt_g(qc, hp, kc)
                if s >= LAG:
                    pqc, php, pkc = steps[s - LAG]
                    av_g(pqc, php, pkc)
                    if pkc == 4 * (pqc + 1) - 1:
                        rec_due.append(s + 1)
                        den_due.append(s + 3)
                if rec_due and s >= rec_due[0]:
                    rec_due.pop(0)
                    flush_rec()
                if den_due and s >= den_due[0]:
                    den_due.pop(0)
                    flush_den()
                sl = slot0 if qc == 0 else slot1
                if kc in sl:
                    if kc == max(sl):
                        flush_fins()  # vp writes before trailing A@V reads
                    pop_fill(sl[kc])
            # drain the lag (the last group's A@V + pu copy)
            for (qc, hp, kc) in steps[-LAG:]:
                av_g(qc, hp, kc)

            # ---- tail: remaining fillers + SE st4-7 as half units; the
            # last group's denominator chain hides behind SE st4/st5
            # partial accumulations (dc 0-4 emitted before the final
            # divisions, the outT1[:,5] chunk after) ----
            while fillers:
                pop_fill(1)
            flush_fins()

            tail_ps = {}
            tail_ysb = {}

            def SEt_head(st):
                tail_ysb[st] = ysbp.tile([128, D], BF16, tag="ysb",
                                         name=f"ysbt{st}")
                ps = psb.tile([128, 1024], F32, tag="big", name=f"set{st}")
                tail_ps[st] = ps
                oT = outTq[st // 4]
                oc = 128 * (st % 4)
                for c0, w in ((0, 512), (512, 256)):
                    for dc in range(NT - 1):
                        nc.tensor.matmul(
                            ps[:, c0:c0 + w], oT[:, dc, oc:oc + 128],
                            wp_sb[:, dc, c0:c0 + w],
                            start=(dc == 0), stop=False)

            def SEt_finish(st):
                ps = tail_ps[st]
                ysb = tail_ysb[st]
                oT = outTq[st // 4]
                oc = 128 * (st % 4)
                dc = NT - 1
                for c0, w in ((0, 512), (512, 256)):
                    nc.tensor.matmul(
                        ps[:, c0:c0 + w], oT[:, dc, oc:oc + 128],
                        wp_sb[:, dc, c0:c0 + w],
                        start=False, stop=True)
                # halves finish on different engines (ScalarE is free once
                # the last exp retired) so the copies overlap
                nc.scalar.activation(ysb[:, 0:512], ps[:, 0:512], AF.Copy)
                nc.vector.tensor_copy(out=ysb[:, 512:768], in_=ps[:, 512:768])
                nc.sync.dma_start(y_d[128 * st:128 * (st + 1), :], ysb[:])

            SEt_head(4)          # dc 0-4 partials: hide the pu copy
            flush_rec(pe=True)   # den-broadcast mms + recips for (1,5)
            SEt_head(5)          # more partials: hide the divisions
            flush_den()          # divisions for (1,5)
            SEt_finish(4)
            SEt_finish(5)
            for st in (6, 7):
                SEt_head(st)
                SEt_finish(st)

    nc.compile()
    return nc


def _get_compiled():
    global _compiled
    if _compiled is None:
        _compiled = _build_nc()
    return _compiled


def _shuffle(t):
    # [D, N] -> [128, NT*N]: row 128*dc+p lands at [p, dc*N:...]
    n = t.shape[1]
    return np.ascontiguousarray(
        t.reshape(NT, 128, n).transpose(1, 0, 2).reshape(128, NT * n))


def _shuffle_nt(t):
    # [D, D] -> [128, NT*(NT*128)]: [p, nt, dc, c] = t[128*dc+p, 128*nt+c]
    return np.ascontiguousarray(
        t.reshape(NT, 128, NT, 128).transpose(1, 2, 0, 3).reshape(128, NT * D))


def _prep_inputs(x, W_attn, W_proj):
    import ml_dtypes

    bf16 = ml_dtypes.bfloat16
    x = np.asarray(x, dtype=np.float32)
    W_attn = np.asarray(W_attn, dtype=np.float32)
    W_proj = np.asarray(W_proj, dtype=np.float32)

    xT = np.transpose(x, (0, 2, 1))
    xTs = np.stack([_shuffle(xT[b]) for b in range(B)], axis=0).astype(bf16)
    wq = _shuffle_nt(W_attn[:, 0:D] * np.float32(0.125)).astype(bf16)
    wk = _shuffle_nt(W_attn[:, D:2 * D]).astype(bf16)
    wv = _shuffle(W_attn[:, 2 * D:3 * D]).astype(bf16)
    wp = _shuffle(W_proj).astype(bf16)
    masks = _build_masks()
    return [
        {"xT": xTs[b], "wq": wq, "wk": wk, "wv": wv, "wp": wp, "masks": masks}
        for b in range(B)
    ]


def kernel(x, W_attn, W_proj):
    from concourse.bass_utils import run_bass_kernel_spmd

    nc = _get_compiled()
    in_maps = _prep_inputs(x, W_attn, W_proj)
    res = run_bass_kernel_spmd(nc, in_maps, list(range(NC_)))
    y = np.stack([np.asarray(res.results[b]["y"]) for b in range(B)], axis=0)
    return y.astype(np.float32)


# revision 62
# speedup vs baseline: 1.0773x; 1.0773x over previous
"""Multi-head causal attention (B=8, S=1024, D=768, H=12) on 8 trn2 NeuronCores.

Strategy: data-parallel over batch (one batch element per core, no collectives).

Per-core dataflow (all matmuls bf16 into fp32 PSUM):
  - host passes x^T and all weights pre-cast to bf16; Q^T/K^T via transposed
    projection (W stationary, x^T moving), V via natural projection (x^T
    stationary, W_v moving) -> no on-device transposes.
  - attention as S^T[k,q] = K @ Q^T per head; the two heads of a 128-row
    group go to the two 512-column halves of one PSUM tile (tile_position
    row packing).
  - causal handling: for diagonal-crossing key blocks the fully-masked low
    query columns are skipped in BOTH the QK^T and A@V matmuls (N-width
    trim); exp covers only the valid span of BOTH heads in ONE strided
    activation, and just the [128,2,128] diagonal triangle pair gets a
    single bf16 mask multiply (in place).
  - kc-granular software pipeline: QK^T(kc) ... A@V(kc-2) keeps the PE fed
    while ScalarE exponentiates; Q/K-proj, V-proj and output-proj work units
    are woven into the remaining gaps (matmuls first, PSUM->SBUF finish
    copies deferred to the next slot) so the PE never idles and its p-state
    stays at max clock.
  - startup: DMA order interleaves xT/wv chunk pairs with the nt=0 blocks
    of wk/wq so the stream can start ~10us in; the warm-up V/Q/K units are
    split into two 3-chunk contraction passes (partial to SBUF bf16, then
    in-place add) so the in-order PE is never serialized behind the last
    xT chunk.
  - softmax: exp straight out of PSUM (1/8 scale folded into W_q host-side;
    scores are small, no max-subtraction); denominator free via a ones
    column appended to V (row 64 of the A@V PSUM); fast reciprocal from
    PSUM on DVE; partition broadcast via a K=1 matmul; division on DVE
    into out^T (rows 64-127 via a small tile + partition-shifting DMA).
  - tail: SE(2,3) woven into the qc=1 stream; the last group's denominator
    chain is hidden behind SE st4/st5 partial accumulations; final copies
    and y DMAs split per 512/256 half.
"""
import sys

if "/opt/trn_rl_repo" not in sys.path:
    sys.path.insert(0, "/opt/trn_rl_repo")

import numpy as np

B, S, D, H = 8, 1024, 768, 12
DH = 64
NC_ = 8
NT = D // 128    # 6
ST = S // 128    # 8
QC = S // 512    # 2
# per-head V slot padded to 128 cols (col 64 = softmax-denominator ones,
# 65-127 zero) so the A@V stationary is a full 128-col FWL-eligible load
VPW = H * 128  # 1536

_compiled = None


def _build_masks():
    # [128, 2, 128] lower-tri mask for the diagonal 128x128 block (the
    # triangle is t-independent), duplicated across the two packed heads
    import ml_dtypes

    i = np.arange(128)[:, None]
    j = np.arange(128)[None, :]
    m = (i <= j).astype(np.float32)
    m = np.broadcast_to(m[:, None, :], (128, 2, 128))
    return np.ascontiguousarray(m).astype(ml_dtypes.bfloat16)


def _build_nc():
    import concourse.bass as bass
    import concourse.mybir as mybir
    import concourse.tile as tile
    from concourse import bacc

    F32 = mybir.dt.float32
    F32R = mybir.dt.float32r
    BF16 = mybir.dt.bfloat16
    AF = mybir.ActivationFunctionType
    MULT = mybir.AluOpType.mult
    ADD = mybir.AluOpType.add

    nc = bacc.Bacc("TRN2", target_bir_lowering=False, debug=False)

    # inputs arrive host-reordered to the exact SBUF layout
    xT_d = nc.dram_tensor("xT", [128, NT * S], BF16, kind="ExternalInput")
    wq_d = nc.dram_tensor("wq", [128, NT * D], BF16, kind="ExternalInput")
    wk_d = nc.dram_tensor("wk", [128, NT * D], BF16, kind="ExternalInput")
    wv_d = nc.dram_tensor("wv", [128, NT * D], BF16, kind="ExternalInput")
    wp_d = nc.dram_tensor("wp", [128, NT * D], BF16, kind="ExternalInput")
    mask_d = nc.dram_tensor("masks", [128, 2, 128], BF16,
                            kind="ExternalInput")
    # bf16 output: halves the tail output-DMA drain; host upcasts
    y_d = nc.dram_tensor("y", [S, D], BF16, kind="ExternalOutput")

    with tile.TileContext(nc) as tc:
        with (
            tc.tile_pool(name="static", bufs=1) as static,
            tc.tile_pool(name="pt", bufs=8) as ptp,
            tc.tile_pool(name="pu", bufs=4) as pup,
            tc.tile_pool(name="rr", bufs=4) as rrp,
            tc.tile_pool(name="dv", bufs=4) as dvp,
            tc.tile_pool(name="ysb", bufs=2) as ysbp,
            tc.tile_pool(name="psb", bufs=2, space="PSUM") as psb,
            tc.tile_pool(name="po", bufs=2, space="PSUM") as pop,
            tc.tile_pool(name="psh", bufs=2, space="PSUM") as psh,
        ):
            # ---- persistent SBUF ----
            xT = static.tile([128, NT, S], BF16)
            qT = static.tile([128, NT, S], BF16)
            kT = static.tile([128, NT, S], BF16)
            vp = static.tile([128, ST, VPW], BF16)
            # out^T split per qc half so tail SE units don't pick up false
            # whole-tile dependencies on the last group's division writes
            outT0 = static.tile([128, NT, 512], BF16)
            outT1 = static.tile([128, NT, 512], BF16)
            outTq = (outT0, outT1)
            msk = static.tile([128, 2, 128], BF16)
            wv_sb = static.tile([128, NT, D], BF16)
            # wk/wq nt-blocked: [p, nt, dc, 128] so one DMA lands a full
            # PJ stationary column block
            wk_sb = static.tile([128, NT, NT, 128], BF16)
            wq_sb = static.tile([128, NT, NT, 128], BF16)
            wp_sb = static.tile([128, NT, D], BF16)

            # ones columns of vp ride along A@V as the softmax denominator;
            # the FWL pad (cols 65-127) is zeroed so PSUM partitions 65-127
            # stay clean
            vp_h = vp[:].rearrange("p s (h e) -> p s h e", e=128)
            nc.vector.memset(vp_h[:, :, :, DH + 1:], 0.0)
            nc.gpsimd.memset(vp_h[:, :, :, DH:DH + 1], 1.0)
            # 1x64 ones row at partition 64: K=1 matmul broadcasts the den
            # row of pu to PSUM partitions 0-63 (no DMA round-trip)
            onesb = static.tile([128, 64], F32)
            nc.vector.memset(onesb[:], 1.0)

            # ---- input DMA order (one HWDGE queue saturates HBM; order =
            # consumption order): xT/wv chunk pairs, nt=0 weight blocks
            # early so the stream can start, masks, rest, wp last ----
            def dma_xv(dc):
                nc.sync.dma_start(xT[:, dc, :], xT_d[:, S * dc:S * (dc + 1)])
                nc.sync.dma_start(wv_sb[:, dc, :], wv_d[:, D * dc:D * (dc + 1)])

            dma_xv(0)
            dma_xv(1)
            dma_xv(2)
            nc.sync.dma_start(wk_sb[:, 0, :, :], wk_d[:, 0:D])
            nc.sync.dma_start(wq_sb[:, 0, :, :], wq_d[:, 0:D])
            dma_xv(3)
            dma_xv(4)
            dma_xv(5)
            nc.sync.dma_start(msk[:], mask_d[:])
            for nt in range(1, NT):
                nc.sync.dma_start(wk_sb[:, nt, :, :],
                                  wk_d[:, D * nt:D * (nt + 1)])
                nc.sync.dma_start(wq_sb[:, nt, :, :],
                                  wq_d[:, D * nt:D * (nt + 1)])
            nc.sync.dma_start(
                wp_sb[:].rearrange("p a b -> p (a b)"), wp_d[:, :])

            # ---- work units: emit matmuls now, return a finish closure ----
            def VP(st, half, fin_eng):
                ps = psh.tile([128, 512], F32, tag="psh", name=f"vps{st}h{half}")
                c0 = 512 * half
                w = 512 if half == 0 else 256
                for dc in range(NT):
                    nc.tensor.matmul(
                        ps[:, 0:w], xT[:, dc, 128 * st:128 * (st + 1)],
                        wv_sb[:, dc, c0:c0 + w],
                        start=(dc == 0), stop=(dc == NT - 1))

                dst = vp[:, st, :].rearrange("p (h e) -> p h e", e=128)

                def fin():
                    src = ps[:, 0:w].rearrange("p (h d) -> p h d", d=DH)
                    if half == 0:
                        o = dst[:, 0:8, 0:DH]
                    else:
                        o = dst[:, 8:12, 0:DH]
                    if fin_eng == "s":
                        nc.scalar.activation(o, src, AF.Copy)
                    else:
                        nc.vector.tensor_copy(out=o, in_=src)
                return fin

            def PJ(w_sb, dstT, nt, sc, fin_eng):
                ps = psh.tile([128, 512], F32, tag="psh", name=f"pj{nt}_{sc}")
                for dc in range(NT):
                    nc.tensor.matmul(
                        ps[:], w_sb[:, nt, dc, :],
                        xT[:, dc, 512 * sc:512 * (sc + 1)],
                        start=(dc == 0), stop=(dc == NT - 1))

                def fin():
                    o = dstT[:, nt, 512 * sc:512 * (sc + 1)]
                    if fin_eng == "s":
                        nc.scalar.activation(o, ps[:], AF.Copy)
                    else:
                        nc.vector.tensor_copy(out=o, in_=ps[:])
                return fin

            # ---- split warm-up units: two 3-chunk contraction passes so
            # the PE isn't serialized behind the last xT chunk during the
            # input DMA window. Partial lands in the bf16 destination, the
            # second pass adds the PSUM half in place on DVE. ----
            def VPw_pass(st, half, p):
                ps = psh.tile([128, 512], F32, tag="psh",
                              name=f"vw{st}h{half}p{p}")
                c0 = 512 * half
                w = 512 if half == 0 else 256
                dcs = (0, 1, 2) if p == 0 else (3, 4, 5)
                for i, dc in enumerate(dcs):
                    nc.tensor.matmul(
                        ps[:, 0:w], xT[:, dc, 128 * st:128 * (st + 1)],
                        wv_sb[:, dc, c0:c0 + w],
                        start=(i == 0), stop=(i == 2))
                dst = vp[:, st, :].rearrange("p (h e) -> p h e", e=128)
                src = ps[:, 0:w].rearrange("p (h d) -> p h d", d=DH)
                if half == 0:
                    o = dst[:, 0:8, 0:DH]
                else:
                    o = dst[:, 8:12, 0:DH]
                if p == 0:
                    nc.vector.tensor_copy(out=o, in_=src)
                else:
                    nc.vector.tensor_tensor(o, src, o, ADD)

            def PJw_pass(w_sb, dstT, nt, sc, p):
                ps = psh.tile([128, 512], F32, tag="psh",
                              name=f"pw{nt}_{sc}p{p}")
                dcs = (0, 1, 2) if p == 0 else (3, 4, 5)
                for i, dc in enumerate(dcs):
                    nc.tensor.matmul(
                        ps[:], w_sb[:, nt, dc, :],
                        xT[:, dc, 512 * sc:512 * (sc + 1)],
                        start=(i == 0), stop=(i == 2))
                o = dstT[:, nt, 512 * sc:512 * (sc + 1)]
                if p == 0:
                    nc.vector.tensor_copy(out=o, in_=ps[:])
                else:
                    nc.vector.tensor_tensor(o, ps[:], o, ADD)

            ysb_tiles = {}

            def SE(st, half, fin_eng):
                if half == 0:
                    ysb_tiles[st] = ysbp.tile([128, D], BF16, tag="ysb",
                                              name=f"ysb{st}")
                ysb = ysb_tiles[st]
                ps = psh.tile([128, 512], F32, tag="psh", name=f"se{st}h{half}")
                c0 = 512 * half
                w = 512 if half == 0 else 256
                oT = outTq[st // 4]
                oc = 128 * (st % 4)
                for dc in range(NT):
                    nc.tensor.matmul(
                        ps[:, 0:w], oT[:, dc, oc:oc + 128],
                        wp_sb[:, dc, c0:c0 + w],
                        start=(dc == 0), stop=(dc == NT - 1))

                def fin():
                    if fin_eng == "s":
                        nc.scalar.activation(ysb[:, c0:c0 + w], ps[:, 0:w],
                                             AF.Copy)
                    else:
                        nc.vector.tensor_copy(out=ysb[:, c0:c0 + w],
                                              in_=ps[:, 0:w])
                    nc.sync.dma_start(y_d[128 * st:128 * (st + 1), c0:c0 + w],
                                      ysb[:, c0:c0 + w])
                return fin

            # ---- filler scheduler ----
            fillers = []
            pend_fin = []

            def flush_fins():
                while pend_fin:
                    pend_fin.pop(0)()

            def pop_fill(n=1):
                for _ in range(n):
                    flush_fins()
                    if fillers:
                        fin = fillers.pop(0)[1]()
                        pend_fin.append(fin)

            def force(*keys):
                # deadline path: unit's finish copy must land before the
                # consumer instructions that follow, so emit it immediately
                for key in keys:
                    for i, (k, fn) in enumerate(fillers):
                        if k == key:
                            flush_fins()
                            fillers.pop(i)
                            fn()()
                            break

            def mk_vp(st, half, e):
                return (f"vp{st}h{half}", lambda: VP(st, half, e))

            def mk_pj(kind, nt, sc, e):
                if kind == "q":
                    return (f"q{nt}s{sc}", lambda: PJ(wq_sb, qT, nt, sc, e))
                return (f"k{nt}s{sc}", lambda: PJ(wk_sb, kT, nt, sc, e))

            def mk_se(st, half, e):
                return (f"se{st}h{half}", lambda: SE(st, half, e))

            # ---- attention helpers ----
            pend_rec = []   # (hp, qc, pu): needs den-broadcast mm + recip
            pend_den = []   # (hp, qc, pu, rr0, rr1): needs division

            def flush_rec(pe=False):
                while pend_rec:
                    hp_, qc_, pu_ = pend_rec.pop(0)
                    rr = rrp.tile([64, 1024], F32, tag="rr", name="rr")
                    # K=1 matmul broadcasts the den row of pu to PSUM
                    # partitions 0-63 (no DRAM round-trip)
                    for hh in range(2):
                        denb = psh.tile([128, 512], F32, tag="psh",
                                        name=f"denb{hh}")
                        nc.tensor.matmul(
                            denb[0:64, :], onesb[64:65, :].bitcast(F32R),
                            pu_[64:65, 512 * hh:512 * (hh + 1)],
                            start=True, stop=True, tile_position=(64, 0))
                        nc.vector.reciprocal_approx_fast(
                            out=rr[:, 512 * hh:512 * (hh + 1)],
                            in_=denb[0:64, :])
                    pend_den.append((hp_, qc_, pu_,
                                     rr[:, 0:512], rr[:, 512:1024]))

            def flush_den():
                while pend_den:
                    hp_, qc_, pu_, rr0, rr1 = pend_den.pop(0)
                    oT = outTq[qc_]
                    nc.vector.tensor_tensor(
                        oT[0:64, hp_, :],
                        pu_[0:64, 0:512], rr0, MULT)
                    # DVE lanes are partition-locked: rows 64-127 go via a
                    # small tile + partition-shifting local DMA
                    dv = dvp.tile([64, 512], BF16, tag="dv", name="dv")
                    nc.vector.tensor_tensor(dv[:], pu_[0:64, 512:1024],
                                            rr1, MULT)
                    nc.sync.dma_start(oT[64:128, hp_, :], dv[:])

            # ---- the attention kc-stream ----
            gstate = {}

            def qkt_g(qc, hp, kc):
                g = gstate.setdefault((qc, hp), {"pts": {}, "offs": {},
                                                 "po": {}})
                t = kc - 4 * qc
                off = 128 * t if 0 <= t <= 3 else 0
                g["offs"][kc] = off
                ps = psb.tile([128, 1024], F32, tag="big",
                              name=f"s_{qc}_{hp}_{kc}")
                for hh in range(2):
                    rows = slice(64 * hh, 64 * (hh + 1))
                    nc.tensor.matmul(
                        ps[:, 512 * hh + off:512 * (hh + 1)],
                        kT[rows, hp, 128 * kc:128 * (kc + 1)],
                        qT[rows, hp, 512 * qc + off:512 * (qc + 1)],
                        start=True, stop=True,
                        tile_position=(64 * hh, 0))
                pt = ptp.tile([128, 1024], BF16, tag="pt")
                if 0 <= t <= 3:
                    # one strided activation covers both heads' valid span;
                    # one strided tensor_tensor masks both diagonal triangles
                    pv = pt[:].rearrange("p (h q) -> p h q", h=2)
                    sv = ps[:].rearrange("p (h q) -> p h q", h=2)
                    nc.scalar.activation(pv[:, :, off:512], sv[:, :, off:512],
                                         AF.Exp)
                    tri = slice(off, off + 128)
                    nc.vector.tensor_tensor(
                        pv[:, :, tri], pv[:, :, tri], msk[:], MULT)
                else:
                    nc.scalar.activation(pt[:], ps[:], AF.Exp)
                g["pts"][kc] = [(pt, 0), (pt, 512)]

            def av_g(qc, hp, kc):
                g = gstate[(qc, hp)]
                K = 4 * (qc + 1)
                off = g["offs"][kc]
                for hh in range(2):
                    if hh not in g["po"]:
                        g["po"][hh] = pop.tile([128, 512], F32, tag="po",
                                               name=f"po_{qc}_{hp}_{hh}")
                    h = 2 * hp + hh
                    src, c0 = g["pts"][kc][hh]
                    nc.tensor.matmul(
                        g["po"][hh][:, off:512],
                        vp[:, kc, 128 * h:128 * (h + 1)],
                        src[:, c0 + off:c0 + 512],
                        start=(kc == 0), stop=(kc == K - 1))
                if kc == K - 1:
                    pu = pup.tile([65, 1024], F32R, tag="pu")
                    for hh in range(2):
                        nc.vector.tensor_copy(
                            out=pu[:, 512 * hh:512 * (hh + 1)],
                            in_=g["po"][hh][0:65, :])
                    pend_rec.append((hp, qc, pu))
                    del gstate[(qc, hp)]

            # ---- phase A: split warm-up (VP st0-3 + k00/k01/q00) keeps the
            # PE as busy as the chunked input DMA allows ----
            warm = [("v", 0, 0), ("v", 0, 1), ("k", 0, 0), ("v", 1, 0),
                    ("v", 1, 1), ("q", 0, 0), ("v", 2, 0), ("v", 2, 1),
                    ("k", 0, 1), ("v", 3, 0), ("v", 3, 1)]
            for p in (0, 1):
                for kind, a, b in warm:
                    if kind == "v":
                        VPw_pass(a, b, p)
                    elif kind == "k":
                        PJw_pass(wk_sb, kT, a, b, p)
                    else:
                        PJw_pass(wq_sb, qT, a, b, p)

            # VP units first: their wv/xT inputs land earliest, and (1,0)
            # -- the third group -- already consumes vp st4-7
            fillers += [mk_vp(st, h, "s") for st in (4, 5, 6, 7)
                        for h in (0, 1)]
            for nt in range(1, NT):
                fillers += [mk_pj("q", nt, 0, "s"), mk_pj("k", nt, 0, "s"),
                            mk_pj("k", nt, 1, "s")]
            fillers += [mk_pj("q", nt, 1, "v") for nt in range(NT)]

            # ---- the stream: qc0/qc1 groups interleaved so the ScalarE
            # exp load (3.3us per qc0 group vs 7.9us per qc1 group) is
            # spread evenly -- otherwise the 2-buf score-PSUM pool couples
            # QK^T(s+2) to exp(s) and the PE stalls through the qc1 half ----
            LAG = 3
            slot0 = {0: 1, 1: 1, 2: 1, 3: 1}
            slot1 = {0: 1, 2: 1, 4: 1, 6: 1}
            groups = [(0, 0), (0, 1), (1, 0), (0, 2), (1, 1), (0, 3),
                      (1, 2), (0, 4), (1, 3), (0, 5), (1, 4), (1, 5)]
            steps = [(qc, hp, kc)
                     for qc, hp in groups
                     for kc in range(4 * (qc + 1))]
            rec_due = []   # stream indices: den-broadcast mm + reciprocal
            den_due = []   # stream indices: divisions (2 steps later)
            gi = -1
            for s, (qc, hp, kc) in enumerate(steps):
                if kc == 0:
                    gi += 1
                    if qc == 0 and hp > 0:
                        force(f"q{hp}s0", f"k{hp}s0")
                    elif qc == 1:
                        force(f"q{hp}s0", f"k{hp}s0", f"k{hp}s1",
                              f"q{hp}s1")
                        # A@V kc>=4 reads vp st4-7: make sure the VP
                        # fillers have run (no-op once popped)
                        force(*[f"vp{st}h{h}" for st in (4, 5, 6, 7)
                                for h in (0, 1)])
                    if gi == 10:
                        # all qc0 groups divided by now: SE st0-3 fill the
                        # last two (qc1) groups
                        fillers += [mk_se(st, h, "v") for st in (0, 1, 2, 3)
                                    for h in (0, 1)]
                qkt_g(qc, hp, kc)
                if s >= LAG:
                    pqc, php, pkc = steps[s - LAG]
                    av_g(pqc, php, pkc)
                    if pkc == 4 * (pqc + 1) - 1:
                        rec_due.append(s + 1)
                        den_due.append(s + 3)
                if rec_due and s >= rec_due[0]:
                    rec_due.pop(0)
                    flush_rec()
                if den_due and s >= den_due[0]:
                    den_due.pop(0)
                    flush_den()
                sl = slot0 if qc == 0 else slot1
                if kc in sl:
                    if kc == max(sl):
                        flush_fins()  # vp writes before trailing A@V reads
                    pop_fill(sl[kc])
            # drain the lag (the last group's A@V + pu copy)
            for (qc, hp, kc) in steps[-LAG:]:
                av_g(qc, hp, kc)

            # ---- tail: remaining fillers + SE st4-7 as half units; the
            # last group's denominator chain hides behind SE st4/st5
            # partial accumulations (dc 0-4 emitted before the final
            # divisions, the outT1[:,5] chunk after) ----
            while fillers:
                pop_fill(1)
            flush_fins()

            tail_ps = {}
            tail_ysb = {}

            def SEt_head(st):
                tail_ysb[st] = ysbp.tile([128, D], BF16, tag="ysb",
                                         name=f"ysbt{st}")
                ps = psb.tile([128, 1024], F32, tag="big", name=f"set{st}")
                tail_ps[st] = ps
                oT = outTq[st // 4]
                oc = 128 * (st % 4)
                for c0, w in ((0, 512), (512, 256)):
                    for dc in range(NT - 1):
                        nc.tensor.matmul(
                            ps[:, c0:c0 + w], oT[:, dc, oc:oc + 128],
                            wp_sb[:, dc, c0:c0 + w],
                            start=(dc == 0), stop=False)

            def SEt_finish(st):
                ps = tail_ps[st]
                ysb = tail_ysb[st]
                oT = outTq[st // 4]
                oc = 128 * (st % 4)
                dc = NT - 1
                for c0, w in ((0, 512), (512, 256)):
                    nc.tensor.matmul(
                        ps[:, c0:c0 + w], oT[:, dc, oc:oc + 128],
                        wp_sb[:, dc, c0:c0 + w],
                        start=False, stop=True)
                # halves finish on different engines (ScalarE is free once
                # the last exp retired) so the copies overlap
                nc.scalar.activation(ysb[:, 0:512], ps[:, 0:512], AF.Copy)
                nc.sync.dma_start(y_d[128 * st:128 * (st + 1), 0:512],
                                  ysb[:, 0:512])
                nc.vector.tensor_copy(out=ysb[:, 512:768], in_=ps[:, 512:768])
                nc.sync.dma_start(y_d[128 * st:128 * (st + 1), 512:768],
                                  ysb[:, 512:768])

            SEt_head(4)          # dc 0-4 partials: hide the pu copy
            flush_rec(pe=True)   # den-broadcast mms + recips for (1,5)
            SEt_head(5)          # more partials: hide the divisions
            flush_den()          # divisions for (1,5)
            SEt_finish(4)
            SEt_finish(5)
            for st in (6, 7):
                SEt_head(st)
                SEt_finish(st)

    nc.compile()
    return nc


def _get_compiled():
    global _compiled
    if _compiled is None:
        _compiled = _build_nc()
    return _compiled


def _shuffle(t):
    # [D, N] -> [128, NT*N]: row 128*dc+p lands at [p, dc*N:...]
    n = t.shape[1]
    return np.ascontiguousarray(
        t.reshape(NT, 128, n).transpose(1, 0, 2).reshape(128, NT * n))


def _shuffle_nt(t):
    # [D, D] -> [128, NT*(NT*128)]: [p, nt, dc, c] = t[128*dc+p, 128*nt+c]
    return np.ascontiguousarray(
        t.reshape(NT, 128, NT, 128).transpose(1, 2, 0, 3).reshape(128, NT * D))


def _prep_inputs(x, W_attn, W_proj):
    import ml_dtypes

    bf16 = ml_dtypes.bfloat16
    x = np.asarray(x, dtype=np.float32)
    W_attn = np.asarray(W_attn, dtype=np.float32)
    W_proj = np.asarray(W_proj, dtype=np.float32)

    xT = np.transpose(x, (0, 2, 1))
    xTs = np.stack([_shuffle(xT[b]) for b in range(B)], axis=0).astype(bf16)
    wq = _shuffle_nt(W_attn[:, 0:D] * np.float32(0.125)).astype(bf16)
    wk = _shuffle_nt(W_attn[:, D:2 * D]).astype(bf16)
    wv = _shuffle(W_attn[:, 2 * D:3 * D]).astype(bf16)
    wp = _shuffle(W_proj).astype(bf16)
    masks = _build_masks()
    return [
        {"xT": xTs[b], "wq": wq, "wk": wk, "wv": wv, "wp": wp, "masks": masks}
        for b in range(B)
    ]


def kernel(x, W_attn, W_proj):
    from concourse.bass_utils import run_bass_kernel_spmd

    nc = _get_compiled()
    in_maps = _prep_inputs(x, W_attn, W_proj)
    res = run_bass_kernel_spmd(nc, in_maps, list(range(NC_)))
    y = np.stack([np.asarray(res.results[b]["y"]) for b in range(B)], axis=0)
    return y.astype(np.float32)


# revision 63
# speedup vs baseline: 1.0888x; 1.0107x over previous
"""Multi-head causal attention (B=8, S=1024, D=768, H=12) on 8 trn2 NeuronCores.

Strategy: data-parallel over batch (one batch element per core, no collectives).

Per-core dataflow (all matmuls bf16 into fp32 PSUM):
  - host passes x^T and all weights pre-cast to bf16; Q^T/K^T via transposed
    projection (W stationary, x^T moving), V via natural projection (x^T
    stationary, W_v moving) -> no on-device transposes.
  - attention as S^T[k,q] = K @ Q^T per head; the two heads of a 128-row
    group go to the two 512-column halves of one PSUM tile (tile_position
    row packing).
  - causal handling: for diagonal-crossing key blocks the fully-masked low
    query columns are skipped in BOTH the QK^T and A@V matmuls (N-width
    trim); exp covers only the valid span of BOTH heads in ONE strided
    activation, and just the [128,2,128] diagonal triangle pair gets a
    single bf16 mask multiply (in place).
  - kc-granular software pipeline: QK^T(kc) ... A@V(kc-2) keeps the PE fed
    while ScalarE exponentiates; Q/K-proj, V-proj and output-proj work units
    are woven into the remaining gaps (matmuls first, PSUM->SBUF finish
    copies deferred to the next slot) so the PE never idles and its p-state
    stays at max clock.
  - startup: DMA order interleaves xT/wv chunk pairs with the nt=0 blocks
    of wk/wq so the stream can start ~10us in; the warm-up V/Q/K units are
    split into two 3-chunk contraction passes (partial to SBUF bf16, then
    in-place add) so the in-order PE is never serialized behind the last
    xT chunk.
  - softmax: exp straight out of PSUM (1/8 scale folded into W_q host-side;
    scores are small, no max-subtraction); denominator free via a ones
    column appended to V (row 64 of the A@V PSUM); fast reciprocal from
    PSUM on DVE; partition broadcast via a K=1 matmul; division on DVE
    into out^T (rows 64-127 via a small tile + partition-shifting DMA).
  - tail: SE(2,3) woven into the qc=1 stream; the last group's denominator
    chain is hidden behind SE st4/st5 partial accumulations; final copies
    and y DMAs split per 512/256 half.
"""
import sys

if "/opt/trn_rl_repo" not in sys.path:
    sys.path.insert(0, "/opt/trn_rl_repo")

import numpy as np

B, S, D, H = 8, 1024, 768, 12
DH = 64
NC_ = 8
NT = D // 128    # 6
ST = S // 128    # 8
QC = S // 512    # 2
# per-head V slot padded to 128 cols (col 64 = softmax-denominator ones,
# 65-127 zero) so the A@V stationary is a full 128-col FWL-eligible load
VPW = H * 128  # 1536

_compiled = None


def _build_masks():
    # [128, 2, 128] lower-tri mask for the diagonal 128x128 block (the
    # triangle is t-independent), duplicated across the two packed heads
    import ml_dtypes

    i = np.arange(128)[:, None]
    j = np.arange(128)[None, :]
    m = (i <= j).astype(np.float32)
    m = np.broadcast_to(m[:, None, :], (128, 2, 128))
    return np.ascontiguousarray(m).astype(ml_dtypes.bfloat16)


def _build_nc():
    import concourse.bass as bass
    import concourse.mybir as mybir
    import concourse.tile as tile
    from concourse import bacc

    F32 = mybir.dt.float32
    F32R = mybir.dt.float32r
    BF16 = mybir.dt.bfloat16
    AF = mybir.ActivationFunctionType
    MULT = mybir.AluOpType.mult
    ADD = mybir.AluOpType.add

    nc = bacc.Bacc("TRN2", target_bir_lowering=False, debug=False)

    # inputs arrive host-reordered to the exact SBUF layout
    xT_d = nc.dram_tensor("xT", [128, NT * S], BF16, kind="ExternalInput")
    wq_d = nc.dram_tensor("wq", [128, NT * D], BF16, kind="ExternalInput")
    wk_d = nc.dram_tensor("wk", [128, NT * D], BF16, kind="ExternalInput")
    wv_d = nc.dram_tensor("wv", [128, NT * D], BF16, kind="ExternalInput")
    wp_d = nc.dram_tensor("wp", [128, NT * D], BF16, kind="ExternalInput")
    mask_d = nc.dram_tensor("masks", [128, 2, 128], BF16,
                            kind="ExternalInput")
    # bf16 output: halves the tail output-DMA drain; host upcasts
    y_d = nc.dram_tensor("y", [S, D], BF16, kind="ExternalOutput")

    with tile.TileContext(nc) as tc:
        with (
            tc.tile_pool(name="static", bufs=1) as static,
            tc.tile_pool(name="pt", bufs=8) as ptp,
            tc.tile_pool(name="pu", bufs=4) as pup,
            tc.tile_pool(name="rr", bufs=4) as rrp,
            tc.tile_pool(name="dv", bufs=4) as dvp,
            tc.tile_pool(name="ysb", bufs=2) as ysbp,
            tc.tile_pool(name="psb", bufs=2, space="PSUM") as psb,
            tc.tile_pool(name="po", bufs=2, space="PSUM") as pop,
            tc.tile_pool(name="psh", bufs=2, space="PSUM") as psh,
        ):
            # ---- persistent SBUF ----
            xT = static.tile([128, NT, S], BF16)
            qT = static.tile([128, NT, S], BF16)
            kT = static.tile([128, NT, S], BF16)
            vp = static.tile([128, ST, VPW], BF16)
            # out^T split per qc half so tail SE units don't pick up false
            # whole-tile dependencies on the last group's division writes
            outT0 = static.tile([128, NT, 512], BF16)
            outT1 = static.tile([128, NT, 512], BF16)
            outTq = (outT0, outT1)
            msk = static.tile([128, 2, 128], BF16)
            wv_sb = static.tile([128, NT, D], BF16)
            # wk/wq nt-blocked: [p, nt, dc, 128] so one DMA lands a full
            # PJ stationary column block
            wk_sb = static.tile([128, NT, NT, 128], BF16)
            wq_sb = static.tile([128, NT, NT, 128], BF16)
            wp_sb = static.tile([128, NT, D], BF16)

            # ones columns of vp ride along A@V as the softmax denominator;
            # the FWL pad (cols 65-127) is zeroed so PSUM partitions 65-127
            # stay clean
            vp_h = vp[:].rearrange("p s (h e) -> p s h e", e=128)
            nc.vector.memset(vp_h[:, :, :, DH + 1:], 0.0)
            nc.gpsimd.memset(vp_h[:, :, :, DH:DH + 1], 1.0)
            # 1x64 ones row at partition 64: K=1 matmul broadcasts the den
            # row of pu to PSUM partitions 0-63 (no DMA round-trip)
            onesb = static.tile([128, 64], F32)
            nc.vector.memset(onesb[:], 1.0)

            # ---- input DMA order (one HWDGE queue saturates HBM; order =
            # consumption order): xT/wv chunk pairs, nt=0 weight blocks
            # early so the stream can start, masks, rest, wp last ----
            def dma_xv(dc):
                nc.sync.dma_start(xT[:, dc, :], xT_d[:, S * dc:S * (dc + 1)])
                nc.sync.dma_start(wv_sb[:, dc, :], wv_d[:, D * dc:D * (dc + 1)])

            dma_xv(0)
            dma_xv(1)
            dma_xv(2)
            nc.sync.dma_start(wk_sb[:, 0, :, :], wk_d[:, 0:D])
            nc.sync.dma_start(wq_sb[:, 0, :, :], wq_d[:, 0:D])
            dma_xv(3)
            dma_xv(4)
            dma_xv(5)
            nc.sync.dma_start(msk[:], mask_d[:])
            for nt in range(1, NT):
                nc.sync.dma_start(wk_sb[:, nt, :, :],
                                  wk_d[:, D * nt:D * (nt + 1)])
                nc.sync.dma_start(wq_sb[:, nt, :, :],
                                  wq_d[:, D * nt:D * (nt + 1)])
            nc.sync.dma_start(
                wp_sb[:].rearrange("p a b -> p (a b)"), wp_d[:, :])

            # ---- work units: emit matmuls now, return a finish closure ----
            def VP(st, half, fin_eng):
                ps = psh.tile([128, 512], F32, tag="psh", name=f"vps{st}h{half}")
                c0 = 512 * half
                w = 512 if half == 0 else 256
                for dc in range(NT):
                    nc.tensor.matmul(
                        ps[:, 0:w], xT[:, dc, 128 * st:128 * (st + 1)],
                        wv_sb[:, dc, c0:c0 + w],
                        start=(dc == 0), stop=(dc == NT - 1))

                dst = vp[:, st, :].rearrange("p (h e) -> p h e", e=128)

                def fin():
                    src = ps[:, 0:w].rearrange("p (h d) -> p h d", d=DH)
                    if half == 0:
                        o = dst[:, 0:8, 0:DH]
                    else:
                        o = dst[:, 8:12, 0:DH]
                    if fin_eng == "s":
                        nc.scalar.activation(o, src, AF.Copy)
                    else:
                        nc.vector.tensor_copy(out=o, in_=src)
                return fin

            def PJ(w_sb, dstT, nt, sc, fin_eng):
                ps = psh.tile([128, 512], F32, tag="psh", name=f"pj{nt}_{sc}")
                for dc in range(NT):
                    nc.tensor.matmul(
                        ps[:], w_sb[:, nt, dc, :],
                        xT[:, dc, 512 * sc:512 * (sc + 1)],
                        start=(dc == 0), stop=(dc == NT - 1))

                def fin():
                    o = dstT[:, nt, 512 * sc:512 * (sc + 1)]
                    if fin_eng == "s":
                        nc.scalar.activation(o, ps[:], AF.Copy)
                    else:
                        nc.vector.tensor_copy(out=o, in_=ps[:])
                return fin

            # ---- split warm-up units: two 3-chunk contraction passes so
            # the PE isn't serialized behind the last xT chunk during the
            # input DMA window. Partial lands in the bf16 destination, the
            # second pass adds the PSUM half in place on DVE. ----
            def VPw_pass(st, half, p):
                ps = psh.tile([128, 512], F32, tag="psh",
                              name=f"vw{st}h{half}p{p}")
                c0 = 512 * half
                w = 512 if half == 0 else 256
                dcs = (0, 1, 2) if p == 0 else (3, 4, 5)
                for i, dc in enumerate(dcs):
                    nc.tensor.matmul(
                        ps[:, 0:w], xT[:, dc, 128 * st:128 * (st + 1)],
                        wv_sb[:, dc, c0:c0 + w],
                        start=(i == 0), stop=(i == 2))
                dst = vp[:, st, :].rearrange("p (h e) -> p h e", e=128)
                src = ps[:, 0:w].rearrange("p (h d) -> p h d", d=DH)
                if half == 0:
                    o = dst[:, 0:8, 0:DH]
                else:
                    o = dst[:, 8:12, 0:DH]
                if p == 0:
                    nc.vector.tensor_copy(out=o, in_=src)
                else:
                    nc.vector.tensor_tensor(o, src, o, ADD)

            def PJw_pass(w_sb, dstT, nt, sc, p):
                ps = psh.tile([128, 512], F32, tag="psh",
                              name=f"pw{nt}_{sc}p{p}")
                dcs = (0, 1, 2) if p == 0 else (3, 4, 5)
                for i, dc in enumerate(dcs):
                    nc.tensor.matmul(
                        ps[:], w_sb[:, nt, dc, :],
                        xT[:, dc, 512 * sc:512 * (sc + 1)],
                        start=(i == 0), stop=(i == 2))
                o = dstT[:, nt, 512 * sc:512 * (sc + 1)]
                if p == 0:
                    nc.vector.tensor_copy(out=o, in_=ps[:])
                else:
                    nc.vector.tensor_tensor(o, ps[:], o, ADD)

            ysb_tiles = {}

            def SE(st, half, fin_eng):
                if half == 0:
                    ysb_tiles[st] = ysbp.tile([128, D], BF16, tag="ysb",
                                              name=f"ysb{st}")
                ysb = ysb_tiles[st]
                ps = psh.tile([128, 512], F32, tag="psh", name=f"se{st}h{half}")
                c0 = 512 * half
                w = 512 if half == 0 else 256
                oT = outTq[st // 4]
                oc = 128 * (st % 4)
                for dc in range(NT):
                    nc.tensor.matmul(
                        ps[:, 0:w], oT[:, dc, oc:oc + 128],
                        wp_sb[:, dc, c0:c0 + w],
                        start=(dc == 0), stop=(dc == NT - 1))

                def fin():
                    if fin_eng == "s":
                        nc.scalar.activation(ysb[:, c0:c0 + w], ps[:, 0:w],
                                             AF.Copy)
                    else:
                        nc.vector.tensor_copy(out=ysb[:, c0:c0 + w],
                                              in_=ps[:, 0:w])
                    nc.sync.dma_start(y_d[128 * st:128 * (st + 1), c0:c0 + w],
                                      ysb[:, c0:c0 + w])
                return fin

            # ---- filler scheduler ----
            fillers = []
            pend_fin = []

            def flush_fins():
                while pend_fin:
                    pend_fin.pop(0)()

            def pop_fill(n=1):
                for _ in range(n):
                    flush_fins()
                    if fillers:
                        fin = fillers.pop(0)[1]()
                        pend_fin.append(fin)

            def force(*keys):
                # deadline path: unit's finish copy must land before the
                # consumer instructions that follow, so emit it immediately
                for key in keys:
                    for i, (k, fn) in enumerate(fillers):
                        if k == key:
                            flush_fins()
                            fillers.pop(i)
                            fn()()
                            break

            def mk_vp(st, half, e):
                return (f"vp{st}h{half}", lambda: VP(st, half, e))

            def mk_pj(kind, nt, sc, e):
                if kind == "q":
                    return (f"q{nt}s{sc}", lambda: PJ(wq_sb, qT, nt, sc, e))
                return (f"k{nt}s{sc}", lambda: PJ(wk_sb, kT, nt, sc, e))

            def mk_se(st, half, e):
                return (f"se{st}h{half}", lambda: SE(st, half, e))

            # ---- attention helpers ----
            pend_rec = []   # (hp, qc, pu): needs den-broadcast mm + recip
            pend_den = []   # (hp, qc, pu, rr0, rr1): needs division

            def flush_rec(pe=False):
                while pend_rec:
                    hp_, qc_, pu_ = pend_rec.pop(0)
                    rr = rrp.tile([64, 1024], F32, tag="rr", name="rr")
                    # K=1 matmul broadcasts the den row of pu to PSUM
                    # partitions 0-63 (no DRAM round-trip)
                    for hh in range(2):
                        denb = psh.tile([128, 512], F32, tag="psh",
                                        name=f"denb{hh}")
                        nc.tensor.matmul(
                            denb[0:64, :], onesb[64:65, :].bitcast(F32R),
                            pu_[64:65, 512 * hh:512 * (hh + 1)],
                            start=True, stop=True, tile_position=(64, 0))
                        nc.vector.reciprocal_approx_fast(
                            out=rr[:, 512 * hh:512 * (hh + 1)],
                            in_=denb[0:64, :])
                    pend_den.append((hp_, qc_, pu_,
                                     rr[:, 0:512], rr[:, 512:1024]))

            def flush_den():
                while pend_den:
                    hp_, qc_, pu_, rr0, rr1 = pend_den.pop(0)
                    oT = outTq[qc_]
                    nc.vector.tensor_tensor(
                        oT[0:64, hp_, :],
                        pu_[0:64, 0:512], rr0, MULT)
                    # DVE lanes are partition-locked: rows 64-127 go via a
                    # small tile + partition-shifting local DMA
                    dv = dvp.tile([64, 512], BF16, tag="dv", name="dv")
                    nc.vector.tensor_tensor(dv[:], pu_[0:64, 512:1024],
                                            rr1, MULT)
                    nc.sync.dma_start(oT[64:128, hp_, :], dv[:])

            # ---- the attention kc-stream ----
            gstate = {}

            def qkt_g(qc, hp, kc):
                g = gstate.setdefault((qc, hp), {"pts": {}, "offs": {},
                                                 "po": {}})
                t = kc - 4 * qc
                off = 128 * t if 0 <= t <= 3 else 0
                g["offs"][kc] = off
                ps = psb.tile([128, 1024], F32, tag="big",
                              name=f"s_{qc}_{hp}_{kc}")
                for hh in range(2):
                    rows = slice(64 * hh, 64 * (hh + 1))
                    nc.tensor.matmul(
                        ps[:, 512 * hh + off:512 * (hh + 1)],
                        kT[rows, hp, 128 * kc:128 * (kc + 1)],
                        qT[rows, hp, 512 * qc + off:512 * (qc + 1)],
                        start=True, stop=True,
                        tile_position=(64 * hh, 0))
                pt = ptp.tile([128, 1024], BF16, tag="pt")
                if 0 <= t <= 3:
                    # one strided activation covers both heads' valid span;
                    # one strided tensor_tensor masks both diagonal triangles
                    pv = pt[:].rearrange("p (h q) -> p h q", h=2)
                    sv = ps[:].rearrange("p (h q) -> p h q", h=2)
                    nc.scalar.activation(pv[:, :, off:512], sv[:, :, off:512],
                                         AF.Exp)
                    tri = slice(off, off + 128)
                    nc.vector.tensor_tensor(
                        pv[:, :, tri], pv[:, :, tri], msk[:], MULT)
                else:
                    nc.scalar.activation(pt[:], ps[:], AF.Exp)
                g["pts"][kc] = [(pt, 0), (pt, 512)]

            def av_g(qc, hp, kc):
                g = gstate[(qc, hp)]
                K = 4 * (qc + 1)
                off = g["offs"][kc]
                for hh in range(2):
                    if hh not in g["po"]:
                        g["po"][hh] = pop.tile([128, 512], F32, tag="po",
                                               name=f"po_{qc}_{hp}_{hh}")
                    h = 2 * hp + hh
                    src, c0 = g["pts"][kc][hh]
                    nc.tensor.matmul(
                        g["po"][hh][:, off:512],
                        vp[:, kc, 128 * h:128 * (h + 1)],
                        src[:, c0 + off:c0 + 512],
                        start=(kc == 0), stop=(kc == K - 1))
                if kc == K - 1:
                    pu = pup.tile([65, 1024], F32R, tag="pu")
                    for hh in range(2):
                        nc.vector.tensor_copy(
                            out=pu[:, 512 * hh:512 * (hh + 1)],
                            in_=g["po"][hh][0:65, :])
                    pend_rec.append((hp, qc, pu))
                    del gstate[(qc, hp)]

            # ---- phase A: split warm-up (VP st0-3 + k00/k01/q00) keeps the
            # PE as busy as the chunked input DMA allows ----
            warm = [("v", 0, 0), ("v", 0, 1), ("k", 0, 0), ("v", 1, 0),
                    ("v", 1, 1), ("q", 0, 0), ("v", 2, 0), ("v", 2, 1),
                    ("k", 0, 1), ("v", 3, 0), ("v", 3, 1)]
            for p in (0, 1):
                for kind, a, b in warm:
                    if kind == "v":
                        VPw_pass(a, b, p)
                    elif kind == "k":
                        PJw_pass(wk_sb, kT, a, b, p)
                    else:
                        PJw_pass(wq_sb, qT, a, b, p)

            # VP units first: their wv/xT inputs land earliest, and (1,0)
            # -- the third group -- already consumes vp st4-7
            fillers += [mk_vp(st, h, "s") for st in (4, 5, 6, 7)
                        for h in (0, 1)]
            for nt in range(1, NT):
                fillers += [mk_pj("q", nt, 0, "s"), mk_pj("k", nt, 0, "s"),
                            mk_pj("k", nt, 1, "s")]
            fillers += [mk_pj("q", nt, 1, "v") for nt in range(NT)]

            # ---- the stream: qc0/qc1 groups interleaved so the ScalarE
            # exp load (3.3us per qc0 group vs 7.9us per qc1 group) is
            # spread evenly -- otherwise the 2-buf score-PSUM pool couples
            # QK^T(s+2) to exp(s) and the PE stalls through the qc1 half ----
            LAG = 3
            slot0 = {0: 1, 1: 1, 2: 1, 3: 1}
            slot1 = {0: 1, 2: 1, 4: 1, 6: 1}
            # (1,4) moved ahead of (0,5): its division + partition-shift
            # DMA then retire during (0,5), so the tail SE st4-7 partials
            # never wait on outT1[:,4]
            groups = [(0, 0), (0, 1), (1, 0), (0, 2), (1, 1), (0, 3),
                      (1, 2), (0, 4), (1, 3), (1, 4), (0, 5), (1, 5)]
            steps = [(qc, hp, kc)
                     for qc, hp in groups
                     for kc in range(4 * (qc + 1))]
            rec_due = []   # stream indices: den-broadcast mm + reciprocal
            den_due = []   # stream indices: divisions (2 steps later)
            gi = -1
            for s, (qc, hp, kc) in enumerate(steps):
                if kc == 0:
                    gi += 1
                    if qc == 0 and hp > 0:
                        force(f"q{hp}s0", f"k{hp}s0")
                    elif qc == 1:
                        force(f"q{hp}s0", f"k{hp}s0", f"k{hp}s1",
                              f"q{hp}s1")
                        # A@V kc>=4 reads vp st4-7: make sure the VP
                        # fillers have run (no-op once popped)
                        force(*[f"vp{st}h{h}" for st in (4, 5, 6, 7)
                                for h in (0, 1)])
                    if gi == 11:
                        # all qc0 groups divided by now: SE st0-3 fill the
                        # last group; leftovers drain at tail start
                        fillers += [mk_se(st, h, "v") for st in (0, 1, 2, 3)
                                    for h in (0, 1)]
                qkt_g(qc, hp, kc)
                if s >= LAG:
                    pqc, php, pkc = steps[s - LAG]
                    av_g(pqc, php, pkc)
                    if pkc == 4 * (pqc + 1) - 1:
                        rec_due.append(s + 1)
                        den_due.append(s + 3)
                if rec_due and s >= rec_due[0]:
                    rec_due.pop(0)
                    flush_rec()
                if den_due and s >= den_due[0]:
                    den_due.pop(0)
                    flush_den()
                sl = slot0 if qc == 0 else slot1
                if kc in sl:
                    if kc == max(sl):
                        flush_fins()  # vp writes before trailing A@V reads
                    pop_fill(sl[kc])
            # drain the lag (the last group's A@V + pu copy)
            for (qc, hp, kc) in steps[-LAG:]:
                av_g(qc, hp, kc)

            # ---- tail: remaining fillers + SE st4-7 as half units; the
            # last group's denominator chain hides behind SE st4/st5
            # partial accumulations (dc 0-4 emitted before the final
            # divisions, the outT1[:,5] chunk after) ----
            while fillers:
                pop_fill(1)
            flush_fins()

            tail_ps = {}
            tail_ysb = {}

            def SEt_head(st):
                tail_ysb[st] = ysbp.tile([128, D], BF16, tag="ysb",
                                         name=f"ysbt{st}")
                ps = psb.tile([128, 1024], F32, tag="big", name=f"set{st}")
                tail_ps[st] = ps
                oT = outTq[st // 4]
                oc = 128 * (st % 4)
                for c0, w in ((0, 512), (512, 256)):
                    for dc in range(NT - 1):
                        nc.tensor.matmul(
                            ps[:, c0:c0 + w], oT[:, dc, oc:oc + 128],
                            wp_sb[:, dc, c0:c0 + w],
                            start=(dc == 0), stop=False)

            def SEt_finish(st):
                ps = tail_ps[st]
                ysb = tail_ysb[st]
                oT = outTq[st // 4]
                oc = 128 * (st % 4)
                dc = NT - 1
                for c0, w in ((0, 512), (512, 256)):
                    nc.tensor.matmul(
                        ps[:, c0:c0 + w], oT[:, dc, oc:oc + 128],
                        wp_sb[:, dc, c0:c0 + w],
                        start=False, stop=True)
                # halves finish on different engines (ScalarE is free once
                # the last exp retired) so the copies overlap
                nc.scalar.activation(ysb[:, 0:512], ps[:, 0:512], AF.Copy)
                nc.sync.dma_start(y_d[128 * st:128 * (st + 1), 0:512],
                                  ysb[:, 0:512])
                nc.vector.tensor_copy(out=ysb[:, 512:768], in_=ps[:, 512:768])
                nc.sync.dma_start(y_d[128 * st:128 * (st + 1), 512:768],
                                  ysb[:, 512:768])

            SEt_head(4)          # dc 0-4 partials: hide the pu copy
            flush_rec(pe=True)   # den-broadcast mms + recips for (1,5)
            SEt_head(5)          # more partials: hide the divisions
            flush_den()          # divisions for (1,5)
            SEt_finish(4)
            SEt_finish(5)
            for st in (6, 7):
                SEt_head(st)
                SEt_finish(st)

    nc.compile()
    return nc


def _get_compiled():
    global _compiled
    if _compiled is None:
        _compiled = _build_nc()
    return _compiled


def _shuffle(t):
    # [D, N] -> [128, NT*N]: row 128*dc+p lands at [p, dc*N:...]
    n = t.shape[1]
    return np.ascontiguousarray(
        t.reshape(NT, 128, n).transpose(1, 0, 2).reshape(128, NT * n))


def _shuffle_nt(t):
    # [D, D] -> [128, NT*(NT*128)]: [p, nt, dc, c] = t[128*dc+p, 128*nt+c]
    return np.ascontiguousarray(
        t.reshape(NT, 128, NT, 128).transpose(1, 2, 0, 3).reshape(128, NT * D))


def _prep_inputs(x, W_attn, W_proj):
    import ml_dtypes

    bf16 = ml_dtypes.bfloat16
    x = np.asarray(x, dtype=np.float32)
    W_attn = np.asarray(W_attn, dtype=np.float32)
    W_proj = np.asarray(W_proj, dtype=np.float32)

    xT = np.transpose(x, (0, 2, 1))
    xTs = np.stack([_shuffle(xT[b]) for b in range(B)], axis=0).astype(bf16)
    wq = _shuffle_nt(W_attn[:, 0:D] * np.float32(0.125)).astype(bf16)
    wk = _shuffle_nt(W_attn[:, D:2 * D]).astype(bf16)
    wv = _shuffle(W_attn[:, 2 * D:3 * D]).astype(bf16)
    wp = _shuffle(W_proj).astype(bf16)
    masks = _build_masks()
    return [
        {"xT": xTs[b], "wq": wq, "wk": wk, "wv": wv, "wp": wp, "masks": masks}
        for b in range(B)
    ]


def kernel(x, W_attn, W_proj):
    from concourse.bass_utils import run_bass_kernel_spmd

    nc = _get_compiled()
    in_maps = _prep_inputs(x, W_attn, W_proj)
    res = run_bass_kernel_spmd(nc, in_maps, list(range(NC_)))
    y = np.stack([np.asarray(res.results[b]["y"]) for b in range(B)], axis=0)
    return y.astype(np.float32)
